# revision 67
# baseline (speedup 1.0000x reference)
"""Multi-head attention (B=2, S=2048, D=1024, H=16) on 8 TRN2 NeuronCores.

Sharding: core c -> (batch b = c//4, head-group g = c%4 of 4 heads).
Each core computes, for its batch and 4 heads:
    Q/K/V projections, scores softmax (scaled by 1/sqrt(S)), attention
    output, and its partial slice of the output projection.
Host sums the 4 head-group partials per batch.

All device tensors are bf16 (PSUM accumulation stays fp32). The tensor
engine is the roofline (~174us busy); the ACT exp stream (~130us) is
the phase-2 co-pacer, so the schedule keeps both streaming:
  - phase 1 projects only K^T fully plus Q^T's first block (waves chase
    the X^T chunk DMAs; wk chunk 0 + a 4-way-split X^T chunk 0 are
    issued first) -- the attention steps and the exp stream start ~25us
    in; an early dummy exp prefetches the ACT table under the DMAs
  - the remaining Q^T blocks and all of V ride as PSUM-ring "fillers"
    woven between the first two steps' score groups
  - scores S^T [k, q] from lhsT=K^T, rhs=Q^T (K=64 head-pairs packed at
    array rows 0/64 via tile_position); 16 exp groups of 2 chunks per
    step (psS 2x2 banks, psO 2, filler ring 2)
  - exp via ACT with fused 1/sqrt(S) scale, written bf16; a ones-column
    in V~ makes the PV matmul (M=65) also produce softmax denominators
  - the previous step's PV chunks are interleaved 4-at-a-time between
    score-group pairs so neither engine stalls the other
  - normalize O^T columns with 1/z: DVE recip + GPSIMD broadcast + mul
  - output projection per completed query block rides later steps in
    4-slot halves; the final block's jc=0 half accumulates open across
    the last z chain (PE kept warm by dummy matmuls) and closes with
    alternating ACT/DVE copies; y (bf16) streams out via DMA as it is
    produced
"""

import sys

if "/opt/trn_rl_repo" not in sys.path:
    sys.path.insert(0, "/opt/trn_rl_repo")

import numpy as np
import ml_dtypes

B = 2
S = 2048
D = 1024
H = 16
DK = 64
NCORES = 8
HG = 4  # heads per core
J = HG * DK  # 256, per-core projection width
QB = 512  # query block
NQB = S // QB  # 4
NKC = S // 128  # 16 key chunks
NDC = D // 128  # 8 contraction chunks
NJC = J // 128  # 2
SCALE_INV = float(1.0 / np.sqrt(np.float32(S)))

_CACHE = {}
LAST_RESULT = None


def _build():
    import concourse.mybir as mybir
    import concourse.tile as tile
    from concourse import bacc

    f32 = mybir.dt.float32
    bf16 = mybir.dt.bfloat16

    nc = bacc.Bacc("TRN2", target_bir_lowering=False, debug=False)

    xt_d = nc.declare_dram_parameter("xt", [D, S], bf16, isOutput=False)
    wq_d = nc.declare_dram_parameter("wq", [D, J], bf16, isOutput=False)
    wk_d = nc.declare_dram_parameter("wk", [D, J], bf16, isOutput=False)
    wv_d = nc.declare_dram_parameter("wv", [D, J], bf16, isOutput=False)
    w0_d = nc.declare_dram_parameter("w0", [J, D], bf16, isOutput=False)
    y_d = nc.declare_dram_parameter("y", [S, D], bf16, isOutput=True)

    with tile.TileContext(nc) as tc:
        with tc.tile_pool(name="persist", bufs=1) as A:
            # persistent tiles
            qt_t = A.tile([128, NJC, S], bf16)  # Q^T  [j, q]
            kt_t = A.tile([128, NJC, S], bf16)  # K^T  [j, k]
            v_t = A.tile([128, NKC, HG, DK + 1], bf16)  # V~ per head + ones
            ot_t = A.tile([128, NJC, S], bf16)  # O^T scaled  [j, q]
            w0_t = A.tile([128, NJC, D], bf16)
            ones_t = A.tile([128, NKC * HG], bf16)
            nc.vector.memset(ones_t, 1.0)
            nc.vector.tensor_copy(out=v_t[:, :, :, DK : DK + 1], in_=ones_t)
            # Prefetch the ACT exp table (~2.7us TABLE_LOAD) during the DMA
            # lead-in so the first real exp op doesn't pay it.
            warm_t = A.tile([128, 8], bf16)
            nc.scalar.activation(
                out=warm_t,
                in_=ones_t[:, 0:8],
                func=mybir.ActivationFunctionType.Exp,
                scale=1.0,
            )

            # ---- phase 1: load X^T / W; project K fully + Q's first block.
            # The rest of Q and all of V are emitted later as "fillers"
            # woven between the attention steps' score groups, so the ACT
            # exp stream starts ~30us earlier and the remaining projection
            # work hides under it.
            xt_t = A.tile([128, NDC, S], bf16)
            wq_t = A.tile([128, NDC, J], bf16)
            wk_t = A.tile([128, NDC, J], bf16)
            wv_t = A.tile([128, NDC, J], bf16)
            wq_src = wq_d.ap().rearrange("(c p) j -> p c j", p=128)
            wk_src = wk_d.ap().rearrange("(c p) j -> p c j", p=128)
            xt_src = xt_d.ap().rearrange("(c p) q -> p c q", p=128)
            # First X^T chunk split 4-ways (spreads across DMA queues so
            # dc=0 arrives ~4x sooner); wk next (K projects first); the
            # rest issued dc-major so arrival stays sequential.
            nc.sync.dma_start(out=wk_t[:, 0], in_=wk_src[:, 0])
            for q4 in range(4):
                sl = slice(q4 * 512, (q4 + 1) * 512)
                nc.sync.dma_start(out=xt_t[:, 0, sl], in_=xt_src[:, 0, sl])
            for dc in range(1, NDC):
                nc.sync.dma_start(out=wk_t[:, dc], in_=wk_src[:, dc])
                nc.sync.dma_start(out=xt_t[:, dc], in_=xt_src[:, dc])
            for dc in range(NDC):
                nc.sync.dma_start(out=wq_t[:, dc], in_=wq_src[:, dc])
            nc.sync.dma_start(
                out=wv_t, in_=wv_d.ap().rearrange("(c p) j -> p c j", p=128)
            )
            # w0 is not needed until the first output-projection block
            # (~halfway through) -- load it last.
            nc.sync.dma_start(
                out=w0_t, in_=w0_d.ap().rearrange("(c p) m -> p c m", p=128)
            )

            with tc.tile_pool(name="ps1", bufs=8, space="PSUM") as psA:
                # Wave 1 fills all 8 banks with the tiles step (0,0)
                # needs FIRST: K^T key-blocks 0-2 (score groups 0-11)
                # plus BOTH Q^T qb=0 tiles -- K and Q0 chase the same
                # X^T DMAs concurrently instead of serially, starting
                # the exp stream ~4us earlier. K^T's key-block 3 (only
                # needed by score group 12, ~12us into the step) follows
                # as a tiny second wave.
                specs1 = [
                    ("k", jc, kb) for jc in range(2) for kb in range(3)
                ] + [("q", jc, 0) for jc in range(2)]
                tiles1 = [
                    psA.tile([128, QB], f32, tag="p1", name=f"p1_{i}")
                    for i in range(8)
                ]
                for dc in range(NDC):
                    for t, (kind, jc, kb) in zip(tiles1, specs1):
                        w_t = wk_t if kind == "k" else wq_t
                        nc.tensor.matmul(
                            t,
                            w_t[:, dc, jc * 128 : (jc + 1) * 128],
                            xt_t[:, dc, kb * QB : (kb + 1) * QB],
                            start=(dc == 0),
                            stop=(dc == NDC - 1),
                        )
                for idx, (t, (kind, jc, kb)) in enumerate(
                    zip(tiles1, specs1)
                ):
                    dst = kt_t if kind == "k" else qt_t
                    o_ap = dst[:, jc, kb * QB : (kb + 1) * QB]
                    if idx % 2 == 0:
                        nc.vector.tensor_copy(out=o_ap, in_=t)
                    else:
                        nc.scalar.activation(
                            out=o_ap,
                            in_=t,
                            func=mybir.ActivationFunctionType.Copy,
                            scale=1.0,
                        )
                tiles2 = [
                    psA.tile([128, QB], f32, tag="p1", name=f"p1_{i}")
                    for i in range(2)
                ]
                for dc in range(NDC):
                    for jc in range(2):
                        nc.tensor.matmul(
                            tiles2[jc],
                            wk_t[:, dc, jc * 128 : (jc + 1) * 128],
                            xt_t[:, dc, 3 * QB : 4 * QB],
                            start=(dc == 0),
                            stop=(dc == NDC - 1),
                        )
                for jc in range(2):
                    o_ap = kt_t[:, jc, 3 * QB : 4 * QB]
                    if jc % 2 == 0:
                        nc.vector.tensor_copy(out=o_ap, in_=tiles2[jc])
                    else:
                        nc.scalar.activation(
                            out=o_ap,
                            in_=tiles2[jc],
                            func=mybir.ActivationFunctionType.Copy,
                            scale=1.0,
                        )

            # ---- phase 2+3: attention with interleaved output proj ----
            # Steps are (qb, hp) head-PAIRS, 8 total. Per step, the 32
            # score chunks (16 kc x 2 heads, interleaved kcA,kcB,...) are
            # row-packed pairs (K=64 at base partitions 0/64 run
            # concurrently at full-array rate). Score PSUM tiles hold 3
            # chunks (6 banks double-buffered) so one ACT exp op covers
            # 1536 elements and the exp stream runs back-to-back. exp
            # results go to a per-step persistent expst tile; the previous
            # step's 32 PV matmuls (dense K=128 full-array work) are
            # emitted as a clump at the start of the next step, in chunk
            # order so expst chunks free up for the incoming exp stream.
            # After normalize of (qb, hp=1), the output projection for
            # that qb rides along (PSUM borrowed from the score pool) and
            # its y slice DMAs out immediately.
            with (
                tc.tile_pool(name="work", bufs=2) as C,
                tc.tile_pool(name="nrm", bufs=2) as Cn,
                tc.tile_pool(name="ytile", bufs=8) as Cy,
                tc.tile_pool(name="dbounce", bufs=2, space="DRAM") as Cd,
                tc.tile_pool(name="ps_s", bufs=2, space="PSUM") as psS,
                tc.tile_pool(name="ps_o", bufs=1, space="PSUM") as psO,
                tc.tile_pool(name="ps_j", bufs=2, space="PSUM") as psJ,
            ):
                NCH = 2 * NKC  # 32 score chunks per step
                # 16 groups of 2 chunks, each exactly one tile_position
                # score pair: psS tiles are 2 banks (2 bufs -> 4), psO 2,
                # filler ring psJ 2.
                GRPS = [(2 * g, 2 * g + 2) for g in range(NCH // 2)]

                def q_filler(jc, qb):
                    """Deferred Q^T projection tile, woven into a step."""

                    def f():
                        ps = psJ.tile([128, QB], f32, tag="pj", name="ps_pj")
                        for dc in range(NDC):
                            nc.tensor.matmul(
                                ps,
                                wq_t[:, dc, jc * 128 : (jc + 1) * 128],
                                xt_t[:, dc, qb * QB : (qb + 1) * QB],
                                start=(dc == 0),
                                stop=(dc == NDC - 1),
                            )
                        nc.vector.tensor_copy(
                            out=qt_t[:, jc, qb * QB : (qb + 1) * QB], in_=ps
                        )

                    return f

                def v_filler(scp):
                    """Deferred V projection for seq chunks 2scp, 2scp+1."""

                    def f():
                        ps = psJ.tile([128, 2, J], f32, tag="pj", name="ps_pj")
                        for half in range(2):
                            sc = 2 * scp + half
                            for dc in range(NDC):
                                nc.tensor.matmul(
                                    ps[:, half],
                                    xt_t[:, dc, sc * 128 : (sc + 1) * 128],
                                    wv_t[:, dc, :],
                                    start=(dc == 0),
                                    stop=(dc == NDC - 1),
                                )
                        for half in range(2):
                            sc = 2 * scp + half
                            nc.vector.tensor_copy(
                                out=v_t[:, sc, :, 0:DK],
                                in_=ps[:, half].rearrange(
                                    "p (h d) -> p h d", h=HG
                                ),
                            )

                    return f

                def emit_pv(qb, hp, expst):
                    """Dense PV clump: 32 K=128 matmuls in chunk order."""
                    ps_oa = psO.tile([128, QB], f32, tag="oa")
                    ps_ob = psO.tile([128, QB], f32, tag="ob")
                    for c in range(NCH):
                        kc, hb = c // 2, c % 2
                        ps_o = ps_oa if hb == 0 else ps_ob
                        nc.tensor.matmul(
                            ps_o[0 : DK + 1, :],
                            v_t[:, kc, 2 * hp + hb, :],
                            expst[:, c, :],
                            start=(kc == 0),
                            stop=(kc == NKC - 1),
                        )
                    return ps_oa, ps_ob

                def emit_normalize(qb, hp, ps_oa, ps_ob):
                    # 1/z via the single-pass DVE reciprocal_approx_fast
                    # (18-bit, plenty for bf16 outputs) reading the PSUM
                    # denominator row directly; broadcast across
                    # partitions on the idle GPSIMD; the final mul also
                    # reads O straight from PSUM. The two halves are
                    # interleaved so their recip/broadcast/mul chains
                    # pipeline across DVE and GPSIMD.
                    q_sl = slice(qb * QB, (qb + 1) * QB)
                    halves = ((0, ps_oa), (64, ps_ob))
                    o_sbs = {}
                    r_bs = {}
                    for p0, ps_o in halves:
                        o_sb = Cn.tile([DK, QB], f32, tag=f"osb{p0}")
                        nc.vector.tensor_copy(o_sb, ps_o[0:DK, :])
                        o_sbs[p0] = o_sb
                        z_sb = Cn.tile([1, QB], f32, tag=f"zs{p0}")
                        nc.vector.tensor_copy(z_sb, ps_o[DK : DK + 1, :])
                        r_sb = Cn.tile([1, QB], f32, tag=f"rs{p0}")
                        nc.vector.reciprocal_approx_fast(out=r_sb, in_=z_sb)
                        r_b = Cn.tile([64, QB], f32, tag=f"rb{p0}")
                        nc.gpsimd.partition_broadcast(r_b, r_sb)
                        r_bs[p0] = r_b
                    for p0, ps_o in halves:
                        nc.vector.tensor_mul(
                            ot_t[p0 : p0 + 64, hp, q_sl],
                            o_sbs[p0],
                            r_bs[p0],
                        )

                def emit_y_out(y_t, qc, mb, split_dma=False):
                    nc.sync.dma_start(
                        out=y_d.ap()[
                            qc * 128 : (qc + 1) * 128,
                            mb * QB : (mb + 1) * QB,
                        ],
                        in_=y_t,
                    )

                def emit_outproj(qb, lo=0, hi=8, copies_on_act=False):
                    """Output projection for (part of) a completed query
                    block: (qc, mb) slots [lo, hi) in groups of 3 using
                    score-pool PSUM; y slices stream to DRAM as they are
                    produced."""
                    slots = [
                        (qb * NQB + qc4, mb)
                        for qc4 in range(QB // 128)
                        for mb in range(D // QB)
                    ][lo:hi]
                    for g0 in range(0, len(slots), 2):
                        grp = slots[g0 : g0 + 2]
                        ps = psS.tile([128, 2, QB], f32, tag="s", name="ps_s")
                        for i, (qc, mb) in enumerate(grp):
                            for jc in range(NJC):
                                nc.tensor.matmul(
                                    ps[:, i],
                                    ot_t[:, jc, qc * 128 : (qc + 1) * 128],
                                    w0_t[:, jc, mb * QB : (mb + 1) * QB],
                                    start=(jc == 0),
                                    stop=(jc == NJC - 1),
                                )
                        for i, (qc, mb) in enumerate(grp):
                            y_t = Cy.tile([128, QB], bf16, tag="yt")
                            if copies_on_act:
                                nc.scalar.activation(
                                    out=y_t,
                                    in_=ps[:, i],
                                    func=mybir.ActivationFunctionType.Copy,
                                    scale=1.0,
                                )
                            else:
                                nc.vector.tensor_copy(out=y_t, in_=ps[:, i])
                            emit_y_out(y_t, qc, mb)

                def emit_outproj_one(qb, k, on_act=False):
                    """Single out-proj slot through the psJ ring (free
                    outside steps 0-1): avoids holding a score-PSUM
                    buffer and its copy never gates the score ring."""
                    qc, mb = qb * NQB + k // 2, k % 2
                    ps = psJ.tile([128, QB], f32, tag="pj", name="ps_pj")
                    for jc in range(NJC):
                        nc.tensor.matmul(
                            ps,
                            ot_t[:, jc, qc * 128 : (qc + 1) * 128],
                            w0_t[:, jc, mb * QB : (mb + 1) * QB],
                            start=(jc == 0),
                            stop=(jc == NJC - 1),
                        )
                    y_t = Cy.tile([128, QB], bf16, tag="yt")
                    if on_act:
                        nc.scalar.activation(
                            out=y_t,
                            in_=ps,
                            func=mybir.ActivationFunctionType.Copy,
                            scale=1.0,
                        )
                    else:
                        nc.vector.tensor_copy(out=y_t, in_=ps)
                    emit_y_out(y_t, qc, mb)

                def emit_outproj_final_open(qb):
                    """jc=0 partial accumulations for the last block's
                    first 6 slots (2 psS tiles x2 + 2 psJ tiles x1).
                    ot[:,0] is ready (normalized a step earlier), so these
                    run while the exp tail / final z chain are in flight,
                    keeping the PE warm."""
                    slots = [
                        (qb * NQB + qc4, mb)
                        for qc4 in range(QB // 128)
                        for mb in range(D // QB)
                    ]
                    ss_tiles = []
                    for g0 in (0, 2):
                        ps = psS.tile([128, 2, QB], f32, tag="s", name="ps_s")
                        for i, (qc, mb) in enumerate(slots[g0 : g0 + 2]):
                            nc.tensor.matmul(
                                ps[:, i],
                                ot_t[:, 0, qc * 128 : (qc + 1) * 128],
                                w0_t[:, 0, mb * QB : (mb + 1) * QB],
                                start=True,
                                stop=False,
                            )
                        ss_tiles.append((ps, slots[g0 : g0 + 2]))
                    return slots, ss_tiles

                def emit_outproj_final_close(slots, ss_tiles):
                    n_cp = [0]

                    def copy_out_row(ps, qc):
                        # per-slot transfers: two 128KB DMAs land on two
                        # queues, halving the final drain vs one 256KB.
                        # Copies alternate ACT/DVE (both idle by now).
                        for i in range(2):
                            y_t = Cy.tile([128, QB], bf16, tag="yt")
                            if n_cp[0] % 2 == 0:
                                nc.scalar.activation(
                                    out=y_t,
                                    in_=ps[:, i],
                                    func=mybir.ActivationFunctionType.Copy,
                                    scale=1.0,
                                )
                            else:
                                nc.vector.tensor_copy(out=y_t, in_=ps[:, i])
                            n_cp[0] += 1
                            nc.sync.dma_start(
                                out=y_d.ap()[
                                    qc * 128 : (qc + 1) * 128,
                                    i * QB : (i + 1) * QB,
                                ],
                                in_=y_t,
                            )

                    # slots 4-7 go through the psJ ring first: no wait on
                    # the score-ring copies, so every close matmul runs
                    # back-to-back at the warm clock
                    qb = slots[0][0] // NQB
                    for k in (4, 5, 6, 7):
                        emit_outproj_one(qb, k, on_act=(k % 2 == 0))
                    for ps, grp in ss_tiles:
                        for i, (qc, mb) in enumerate(grp):
                            nc.tensor.matmul(
                                ps[:, i],
                                ot_t[:, 1, qc * 128 : (qc + 1) * 128],
                                w0_t[:, 1, mb * QB : (mb + 1) * QB],
                                start=False,
                                stop=True,
                            )
                        copy_out_row(ps, grp[0][0])

                def emit_step(
                    qb, hp, prev, ride=None, leftover=None, fillers=None,
                    final=False
                ):
                    """Scores+exp stream for (qb, hp); the previous step's
                    PV chunks are interleaved between score-group PAIRS
                    (blocks of 4 chunks, trailing by one block) so the
                    tensor queue streams without the psS/exp lockstep and
                    PV matmuls run back-to-back. `ride` is an out-proj
                    portion (qb, lo, hi) emitted mid-step. The final step
                    runs its leftovers after the score stream and chases
                    the exp tail with its own PV."""
                    q_sl = slice(qb * QB, (qb + 1) * QB)
                    expst = C.tile([128, NCH, QB], bf16, tag="expst")
                    if prev is not None:
                        pq, php, pexp = prev
                        ps_oa = psO.tile([128, QB], f32, tag="oa")
                        ps_ob = psO.tile([128, QB], f32, tag="ob")

                    def pv_prev(g0, g1):
                        for c in range(g0, g1):
                            kc, hb = c // 2, c % 2
                            ps_o = ps_oa if hb == 0 else ps_ob
                            nc.tensor.matmul(
                                ps_o[0 : DK + 1, :],
                                v_t[:, kc, 2 * php + hb, :],
                                pexp[:, c, :],
                                start=(kc == 0),
                                stop=(kc == NKC - 1),
                            )

                    def score_group(g0, g1):
                        ps = psS.tile([128, 2, QB], f32, tag="s", name="ps_s")
                        for i, c in enumerate(range(g0, g1)):
                            kc, hb = c // 2, c % 2
                            p0 = hb * 64
                            k_sl = slice(kc * 128, (kc + 1) * 128)
                            nc.tensor.matmul(
                                ps[:, i],
                                kt_t[p0 : p0 + 64, hp, k_sl],
                                qt_t[p0 : p0 + 64, hp, q_sl],
                                start=True,
                                stop=True,
                                tile_position=(p0, 0),
                            )
                        nc.scalar.activation(
                            out=expst[:, g0:g1, :],
                            in_=ps[:, 0 : g1 - g0, :],
                            func=mybir.ActivationFunctionType.Exp,
                            scale=SCALE_INV,
                        )

                    NBLK = len(GRPS) // 2  # 8 blocks of 2 groups
                    if not final:
                        for b in range(NBLK):
                            score_group(*GRPS[2 * b])
                            score_group(*GRPS[2 * b + 1])
                            if prev is not None and b >= 1:
                                pv_prev(4 * (b - 1), 4 * b)
                            if ride is not None and b in (1, 3, 5, 7):
                                # one out-proj slot of an already-normalized
                                # block rides per block, through the idle
                                # psJ ring: the score-PSUM ring is never
                                # displaced and the copy is off its path
                                emit_outproj_one(
                                    ride[0], ride[1] + (b - 1) // 2
                                )
                            if fillers:
                                fillers.pop(0)()
                            if fillers:
                                fillers.pop(0)()
                        if prev is not None:
                            pv_prev(4 * (NBLK - 1), NCH)
                            emit_normalize(pq, php, ps_oa, ps_ob)
                        return expst

                    # final step: prev-PV interleaved as usual; leftover
                    # out-proj rides after the score stream where it fills
                    # the window in which the tensor engine would otherwise
                    # wait for the exp tail before self-PV.
                    for b in range(NBLK):
                        score_group(*GRPS[2 * b])
                        score_group(*GRPS[2 * b + 1])
                        if prev is not None and b >= 1:
                            pv_prev(4 * (b - 1), 4 * b)
                    if prev is not None:
                        pv_prev(4 * (NBLK - 1), NCH)
                        emit_normalize(pq, php, ps_oa, ps_ob)
                    # leftover rides fill the exp-tail wait (they must
                    # fully precede the open jc=0 partials: both use the
                    # psS ring and an open accumulation may not be
                    # recycled)
                    for r in leftover or []:
                        emit_outproj(r[0], r[1], r[2], copies_on_act=True)
                    op_state = emit_outproj_final_open(qb)

                    ps_fa = psO.tile([128, QB], f32, tag="oa")
                    ps_fb = psO.tile([128, QB], f32, tag="ob")
                    for c in range(NCH):
                        kc, hb = c // 2, c % 2
                        ps_o = ps_fa if hb == 0 else ps_fb
                        nc.tensor.matmul(
                            ps_o[0 : DK + 1, :],
                            v_t[:, kc, 2 * hp + hb, :],
                            expst[:, c, :],
                            start=(kc == 0),
                            stop=(kc == NKC - 1),
                        )
                    emit_normalize(qb, hp, ps_fa, ps_fb)
                    # keep the PE p-state warm through the z chain --
                    # idle >1.5us drops the clock and the closing matmuls
                    # would run ~2x slower
                    warm_ps = psJ.tile([128, QB], f32, tag="pj", name="ps_pj")
                    for _ in range(6):
                        nc.tensor.matmul(
                            warm_ps,
                            kt_t[:, 0, 0:128],
                            kt_t[:, 0, 0:QB],
                            start=True,
                            stop=True,
                        )
                    # the last block's jc=1 halves close out after the
                    # final z chain
                    emit_outproj_final_close(*op_state)
                    return expst

                steps = [(qb, hp) for qb in range(NQB) for hp in range(2)]
                # Deferred projection fillers: V seq-chunk pairs 0-1 and
                # the remaining Q blocks ride step 0; V pairs 2-7 ride
                # step 1 (pair p lands before the prev-PV chunks that
                # read v_t[2p] with >=2 group slots of margin).
                step_fillers = {
                    0: [v_filler(0), v_filler(1)]
                    + [q_filler(jc, qb) for qb in (1, 2, 3) for jc in (0, 1)],
                    1: [v_filler(p) for p in range(2, 8)],
                }
                prev = None
                rides = []  # (qb, lo, hi) out-proj portions awaiting a step
                for i, (qb, hp) in enumerate(steps):
                    last = i == len(steps) - 1
                    expst = emit_step(
                        qb,
                        hp,
                        prev,
                        ride=(rides.pop(0) if rides and not last else None),
                        leftover=rides if last else None,
                        fillers=step_fillers.get(i),
                        final=last,
                    )
                    # prev (consumed this step) is normalized at this
                    # step's end; its out-proj rides later steps in halves.
                    if prev is not None and prev[1] == 1:
                        rides.append((prev[0], 0, 4))
                        rides.append((prev[0], 4, 8))
                    prev = (qb, hp, expst)

    nc.compile()
    return nc


def kernel(X, W_Q, W_K, W_V, W_0):
    global LAST_RESULT
    from concourse.bass_utils import run_bass_kernel_spmd
    import os

    bf = ml_dtypes.bfloat16
    X = np.asarray(X, dtype=np.float32)
    W_Q = np.asarray(W_Q, dtype=np.float32).astype(bf)
    W_K = np.asarray(W_K, dtype=np.float32).astype(bf)
    W_V = np.asarray(W_V, dtype=np.float32).astype(bf)
    W_0 = np.asarray(W_0, dtype=np.float32).astype(bf)

    if "nc" not in _CACHE:
        _CACHE["nc"] = _build()
    nc = _CACHE["nc"]

    xt = [np.ascontiguousarray(X[b].T).astype(bf) for b in range(B)]
    in_maps = []
    for c in range(NCORES):
        b, g = c // HG, c % HG
        js = slice(g * J, (g + 1) * J)
        in_maps.append(
            {
                "xt": xt[b],
                "wq": np.ascontiguousarray(W_Q[:, js]),
                "wk": np.ascontiguousarray(W_K[:, js]),
                "wv": np.ascontiguousarray(W_V[:, js]),
                "w0": np.ascontiguousarray(W_0[js, :]),
            }
        )

    trace = bool(int(os.environ.get("KERNEL_TRACE", "0")))
    res = run_bass_kernel_spmd(
        nc, in_maps, list(range(NCORES)), trace=trace
    )
    LAST_RESULT = res

    out = np.zeros((B, S, D), dtype=np.float32)
    for c in range(NCORES):
        out[c // HG] += res.results[c]["y"].astype(np.float32)
    return out



# revision 70
# speedup vs baseline: 1.0139x; 1.0139x over previous
"""Multi-head attention (B=2, S=2048, D=1024, H=16) on 8 TRN2 NeuronCores.

Sharding: core c -> (batch b = c//4, head-group g = c%4 of 4 heads).
Each core computes, for its batch and 4 heads:
    Q/K/V projections, scores softmax (scaled by 1/sqrt(S)), attention
    output, and its partial slice of the output projection.
Host sums the 4 head-group partials per batch.

All device tensors are bf16 (PSUM accumulation stays fp32). The tensor
engine is the roofline (~174us busy); the ACT exp stream (~130us) is
the phase-2 co-pacer, so the schedule keeps both streaming:
  - phase 1 projects only K^T fully plus Q^T's first block (waves chase
    the X^T chunk DMAs; wk chunk 0 + a 4-way-split X^T chunk 0 are
    issued first) -- the attention steps and the exp stream start ~25us
    in; an early dummy exp prefetches the ACT table under the DMAs
  - the remaining Q^T blocks and all of V ride as PSUM-ring "fillers"
    woven between the first two steps' score groups
  - scores S^T [k, q] from lhsT=K^T, rhs=Q^T (K=64 head-pairs packed at
    array rows 0/64 via tile_position); 16 exp groups of 2 chunks per
    step (psS 2x2 banks, psO 2, filler ring 2)
  - exp via ACT with fused 1/sqrt(S) scale, written bf16; a ones-column
    in V~ makes the PV matmul (M=65) also produce softmax denominators
  - the previous step's PV chunks are interleaved 4-at-a-time between
    score-group pairs so neither engine stalls the other
  - normalize O^T columns with 1/z: DVE recip + GPSIMD broadcast + mul
  - output projection per completed query block rides later steps in
    4-slot halves; the final block's jc=0 half accumulates open across
    the last z chain (PE kept warm by dummy matmuls) and closes with
    alternating ACT/DVE copies; y (bf16) streams out via DMA as it is
    produced
"""

import sys

if "/opt/trn_rl_repo" not in sys.path:
    sys.path.insert(0, "/opt/trn_rl_repo")

import numpy as np
import ml_dtypes

B = 2
S = 2048
D = 1024
H = 16
DK = 64
NCORES = 8
HG = 4  # heads per core
J = HG * DK  # 256, per-core projection width
QB = 512  # query block
NQB = S // QB  # 4
NKC = S // 128  # 16 key chunks
NDC = D // 128  # 8 contraction chunks
NJC = J // 128  # 2
SCALE_INV = float(1.0 / np.sqrt(np.float32(S)))

_CACHE = {}
LAST_RESULT = None


def _build():
    import concourse.mybir as mybir
    import concourse.tile as tile
    from concourse import bacc

    f32 = mybir.dt.float32
    bf16 = mybir.dt.bfloat16

    nc = bacc.Bacc("TRN2", target_bir_lowering=False, debug=False)

    xt_d = nc.declare_dram_parameter("xt", [D, S], bf16, isOutput=False)
    wq_d = nc.declare_dram_parameter("wq", [D, J], bf16, isOutput=False)
    wk_d = nc.declare_dram_parameter("wk", [D, J], bf16, isOutput=False)
    wv_d = nc.declare_dram_parameter("wv", [D, J], bf16, isOutput=False)
    w0_d = nc.declare_dram_parameter("w0", [J, D], bf16, isOutput=False)
    y_d = nc.declare_dram_parameter("y", [S, D], bf16, isOutput=True)

    with tile.TileContext(nc) as tc:
        with tc.tile_pool(name="persist", bufs=1) as A:
            # persistent tiles
            qt_t = A.tile([128, NJC, S], bf16)  # Q^T  [j, q]
            kt_t = A.tile([128, NJC, S], bf16)  # K^T  [j, k]
            v_t = A.tile([128, NKC, HG, DK + 1], bf16)  # V~ per head + ones
            ot_t = A.tile([128, NJC, S], bf16)  # O^T scaled  [j, q]
            w0_t = A.tile([128, NJC, D], bf16)
            ones_t = A.tile([128, NKC * HG], bf16)
            nc.vector.memset(ones_t, 1.0)
            nc.vector.tensor_copy(out=v_t[:, :, :, DK : DK + 1], in_=ones_t)
            # Prefetch the ACT exp table (~2.7us TABLE_LOAD) during the DMA
            # lead-in so the first real exp op doesn't pay it.
            warm_t = A.tile([128, 8], bf16)
            nc.scalar.activation(
                out=warm_t,
                in_=ones_t[:, 0:8],
                func=mybir.ActivationFunctionType.Exp,
                scale=1.0,
            )

            # ---- phase 1: load X^T / W; project K fully + Q's first block.
            # The rest of Q and all of V are emitted later as "fillers"
            # woven between the attention steps' score groups, so the ACT
            # exp stream starts ~30us earlier and the remaining projection
            # work hides under it.
            xt_t = A.tile([128, NDC, S], bf16)
            wq_t = A.tile([128, NDC, J], bf16)
            wk_t = A.tile([128, NDC, J], bf16)
            wv_t = A.tile([128, NDC, J], bf16)
            wq_src = wq_d.ap().rearrange("(c p) j -> p c j", p=128)
            wk_src = wk_d.ap().rearrange("(c p) j -> p c j", p=128)
            xt_src = xt_d.ap().rearrange("(c p) q -> p c q", p=128)
            # First X^T chunk split 4-ways (spreads across DMA queues so
            # dc=0 arrives ~4x sooner); wk next (K projects first); the
            # rest issued dc-major so arrival stays sequential.
            nc.sync.dma_start(out=wk_t[:, 0], in_=wk_src[:, 0])
            for q4 in range(4):
                sl = slice(q4 * 512, (q4 + 1) * 512)
                nc.sync.dma_start(out=xt_t[:, 0, sl], in_=xt_src[:, 0, sl])
            for dc in range(1, NDC):
                nc.sync.dma_start(out=wk_t[:, dc], in_=wk_src[:, dc])
                nc.sync.dma_start(out=xt_t[:, dc], in_=xt_src[:, dc])
            for dc in range(NDC):
                nc.sync.dma_start(out=wq_t[:, dc], in_=wq_src[:, dc])
            nc.sync.dma_start(
                out=wv_t, in_=wv_d.ap().rearrange("(c p) j -> p c j", p=128)
            )
            # w0 is not needed until the first output-projection block
            # (~halfway through) -- load it last.
            nc.sync.dma_start(
                out=w0_t, in_=w0_d.ap().rearrange("(c p) m -> p c m", p=128)
            )

            with tc.tile_pool(name="ps1", bufs=8, space="PSUM") as psA:
                # Wave 1 fills all 8 banks with the tiles step (0,0)
                # needs FIRST: K^T key-blocks 0-2 (score groups 0-11)
                # plus BOTH Q^T qb=0 tiles -- K and Q0 chase the same
                # X^T DMAs concurrently instead of serially, starting
                # the exp stream ~4us earlier. K^T's key-block 3 (only
                # needed by score group 12, ~12us into the step) follows
                # as a tiny second wave.
                specs1 = [
                    ("k", jc, kb) for jc in range(2) for kb in range(3)
                ] + [("q", jc, 0) for jc in range(2)]
                tiles1 = [
                    psA.tile([128, QB], f32, tag="p1", name=f"p1_{i}")
                    for i in range(8)
                ]
                for dc in range(NDC):
                    for t, (kind, jc, kb) in zip(tiles1, specs1):
                        w_t = wk_t if kind == "k" else wq_t
                        nc.tensor.matmul(
                            t,
                            w_t[:, dc, jc * 128 : (jc + 1) * 128],
                            xt_t[:, dc, kb * QB : (kb + 1) * QB],
                            start=(dc == 0),
                            stop=(dc == NDC - 1),
                        )
                for idx, (t, (kind, jc, kb)) in enumerate(
                    zip(tiles1, specs1)
                ):
                    dst = kt_t if kind == "k" else qt_t
                    o_ap = dst[:, jc, kb * QB : (kb + 1) * QB]
                    if idx % 2 == 0:
                        nc.vector.tensor_copy(out=o_ap, in_=t)
                    else:
                        nc.scalar.activation(
                            out=o_ap,
                            in_=t,
                            func=mybir.ActivationFunctionType.Copy,
                            scale=1.0,
                        )


            # ---- phase 2+3: attention with interleaved output proj ----
            # Steps are (qb, hp) head-PAIRS, 8 total. Per step, the 32
            # score chunks (16 kc x 2 heads, interleaved kcA,kcB,...) are
            # row-packed pairs (K=64 at base partitions 0/64 run
            # concurrently at full-array rate). Score PSUM tiles hold 3
            # chunks (6 banks double-buffered) so one ACT exp op covers
            # 1536 elements and the exp stream runs back-to-back. exp
            # results go to a per-step persistent expst tile; the previous
            # step's 32 PV matmuls (dense K=128 full-array work) are
            # emitted as a clump at the start of the next step, in chunk
            # order so expst chunks free up for the incoming exp stream.
            # After normalize of (qb, hp=1), the output projection for
            # that qb rides along (PSUM borrowed from the score pool) and
            # its y slice DMAs out immediately.
            with (
                tc.tile_pool(name="work", bufs=2) as C,
                tc.tile_pool(name="nrm", bufs=2) as Cn,
                tc.tile_pool(name="ytile", bufs=8) as Cy,
                tc.tile_pool(name="dbounce", bufs=2, space="DRAM") as Cd,
                tc.tile_pool(name="ps_s", bufs=2, space="PSUM") as psS,
                tc.tile_pool(name="ps_o", bufs=1, space="PSUM") as psO,
                tc.tile_pool(name="ps_j", bufs=2, space="PSUM") as psJ,
            ):
                NCH = 2 * NKC  # 32 score chunks per step
                # 16 groups of 2 chunks, each exactly one tile_position
                # score pair: psS tiles are 2 banks (2 bufs -> 4), psO 2,
                # filler ring psJ 2.
                GRPS = [(2 * g, 2 * g + 2) for g in range(NCH // 2)]

                def q_filler(jc, qb):
                    """Deferred Q^T projection tile, woven into a step."""

                    def f():
                        ps = psJ.tile([128, QB], f32, tag="pj", name="ps_pj")
                        for dc in range(NDC):
                            nc.tensor.matmul(
                                ps,
                                wq_t[:, dc, jc * 128 : (jc + 1) * 128],
                                xt_t[:, dc, qb * QB : (qb + 1) * QB],
                                start=(dc == 0),
                                stop=(dc == NDC - 1),
                            )
                        nc.vector.tensor_copy(
                            out=qt_t[:, jc, qb * QB : (qb + 1) * QB], in_=ps
                        )

                    return f

                def k_filler(jc):
                    """K^T key-block 3 tile, woven into step 0 (score
                    group 12 is its first reader, ~12us into the step)."""

                    def f():
                        ps = psJ.tile([128, QB], f32, tag="pj", name="ps_pj")
                        for dc in range(NDC):
                            nc.tensor.matmul(
                                ps,
                                wk_t[:, dc, jc * 128 : (jc + 1) * 128],
                                xt_t[:, dc, 3 * QB : 4 * QB],
                                start=(dc == 0),
                                stop=(dc == NDC - 1),
                            )
                        nc.vector.tensor_copy(
                            out=kt_t[:, jc, 3 * QB : 4 * QB], in_=ps
                        )

                    return f

                def v_filler(scp):
                    """Deferred V projection for seq chunks 2scp, 2scp+1."""

                    def f():
                        ps = psJ.tile([128, 2, J], f32, tag="pj", name="ps_pj")
                        for half in range(2):
                            sc = 2 * scp + half
                            for dc in range(NDC):
                                nc.tensor.matmul(
                                    ps[:, half],
                                    xt_t[:, dc, sc * 128 : (sc + 1) * 128],
                                    wv_t[:, dc, :],
                                    start=(dc == 0),
                                    stop=(dc == NDC - 1),
                                )
                        for half in range(2):
                            sc = 2 * scp + half
                            nc.vector.tensor_copy(
                                out=v_t[:, sc, :, 0:DK],
                                in_=ps[:, half].rearrange(
                                    "p (h d) -> p h d", h=HG
                                ),
                            )

                    return f

                def emit_pv(qb, hp, expst):
                    """Dense PV clump: 32 K=128 matmuls in chunk order."""
                    ps_oa = psO.tile([128, QB], f32, tag="oa")
                    ps_ob = psO.tile([128, QB], f32, tag="ob")
                    for c in range(NCH):
                        kc, hb = c // 2, c % 2
                        ps_o = ps_oa if hb == 0 else ps_ob
                        nc.tensor.matmul(
                            ps_o[0 : DK + 1, :],
                            v_t[:, kc, 2 * hp + hb, :],
                            expst[:, c, :],
                            start=(kc == 0),
                            stop=(kc == NKC - 1),
                        )
                    return ps_oa, ps_ob

                def emit_normalize(qb, hp, ps_oa, ps_ob):
                    # 1/z via the single-pass DVE reciprocal_approx_fast
                    # (18-bit, plenty for bf16 outputs) reading the PSUM
                    # denominator row directly; broadcast across
                    # partitions on the idle GPSIMD; the final mul also
                    # reads O straight from PSUM. The two halves are
                    # interleaved so their recip/broadcast/mul chains
                    # pipeline across DVE and GPSIMD.
                    q_sl = slice(qb * QB, (qb + 1) * QB)
                    halves = ((0, ps_oa), (64, ps_ob))
                    o_sbs = {}
                    r_bs = {}
                    for p0, ps_o in halves:
                        o_sb = Cn.tile([DK, QB], f32, tag=f"osb{p0}")
                        nc.vector.tensor_copy(o_sb, ps_o[0:DK, :])
                        o_sbs[p0] = o_sb
                        z_sb = Cn.tile([1, QB], f32, tag=f"zs{p0}")
                        nc.vector.tensor_copy(z_sb, ps_o[DK : DK + 1, :])
                        r_sb = Cn.tile([1, QB], f32, tag=f"rs{p0}")
                        nc.vector.reciprocal_approx_fast(out=r_sb, in_=z_sb)
                        r_b = Cn.tile([64, QB], f32, tag=f"rb{p0}")
                        nc.gpsimd.partition_broadcast(r_b, r_sb)
                        r_bs[p0] = r_b
                    for p0, ps_o in halves:
                        nc.vector.tensor_mul(
                            ot_t[p0 : p0 + 64, hp, q_sl],
                            o_sbs[p0],
                            r_bs[p0],
                        )

                def emit_y_out(y_t, qc, mb, split_dma=False):
                    nc.sync.dma_start(
                        out=y_d.ap()[
                            qc * 128 : (qc + 1) * 128,
                            mb * QB : (mb + 1) * QB,
                        ],
                        in_=y_t,
                    )

                def emit_outproj(qb, lo=0, hi=8, copies_on_act=False):
                    """Output projection for (part of) a completed query
                    block: (qc, mb) slots [lo, hi) in groups of 3 using
                    score-pool PSUM; y slices stream to DRAM as they are
                    produced."""
                    slots = [
                        (qb * NQB + qc4, mb)
                        for qc4 in range(QB // 128)
                        for mb in range(D // QB)
                    ][lo:hi]
                    for g0 in range(0, len(slots), 2):
                        grp = slots[g0 : g0 + 2]
                        ps = psS.tile([128, 2, QB], f32, tag="s", name="ps_s")
                        for i, (qc, mb) in enumerate(grp):
                            for jc in range(NJC):
                                nc.tensor.matmul(
                                    ps[:, i],
                                    ot_t[:, jc, qc * 128 : (qc + 1) * 128],
                                    w0_t[:, jc, mb * QB : (mb + 1) * QB],
                                    start=(jc == 0),
                                    stop=(jc == NJC - 1),
                                )
                        for i, (qc, mb) in enumerate(grp):
                            y_t = Cy.tile([128, QB], bf16, tag="yt")
                            if copies_on_act:
                                nc.scalar.activation(
                                    out=y_t,
                                    in_=ps[:, i],
                                    func=mybir.ActivationFunctionType.Copy,
                                    scale=1.0,
                                )
                            else:
                                nc.vector.tensor_copy(out=y_t, in_=ps[:, i])
                            emit_y_out(y_t, qc, mb)

                def emit_outproj_one(qb, k, on_act=False):
                    """Single out-proj slot through the psJ ring (free
                    outside steps 0-1): avoids holding a score-PSUM
                    buffer and its copy never gates the score ring."""
                    qc, mb = qb * NQB + k // 2, k % 2
                    ps = psJ.tile([128, QB], f32, tag="pj", name="ps_pj")
                    for jc in range(NJC):
                        nc.tensor.matmul(
                            ps,
                            ot_t[:, jc, qc * 128 : (qc + 1) * 128],
                            w0_t[:, jc, mb * QB : (mb + 1) * QB],
                            start=(jc == 0),
                            stop=(jc == NJC - 1),
                        )
                    y_t = Cy.tile([128, QB], bf16, tag="yt")
                    if on_act:
                        nc.scalar.activation(
                            out=y_t,
                            in_=ps,
                            func=mybir.ActivationFunctionType.Copy,
                            scale=1.0,
                        )
                    else:
                        nc.vector.tensor_copy(out=y_t, in_=ps)
                    emit_y_out(y_t, qc, mb)

                def emit_outproj_final_open(qb):
                    """jc=0 partial accumulations for the last block's
                    first 6 slots (2 psS tiles x2 + 2 psJ tiles x1).
                    ot[:,0] is ready (normalized a step earlier), so these
                    run while the exp tail / final z chain are in flight,
                    keeping the PE warm."""
                    slots = [
                        (qb * NQB + qc4, mb)
                        for qc4 in range(QB // 128)
                        for mb in range(D // QB)
                    ]
                    ss_tiles = []
                    for g0 in (0, 2):
                        ps = psS.tile([128, 2, QB], f32, tag="s", name="ps_s")
                        for i, (qc, mb) in enumerate(slots[g0 : g0 + 2]):
                            nc.tensor.matmul(
                                ps[:, i],
                                ot_t[:, 0, qc * 128 : (qc + 1) * 128],
                                w0_t[:, 0, mb * QB : (mb + 1) * QB],
                                start=True,
                                stop=False,
                            )
                        ss_tiles.append((ps, slots[g0 : g0 + 2]))
                    return slots, ss_tiles

                def emit_outproj_final_close(slots, ss_tiles):
                    n_cp = [0]

                    def copy_out_row(ps, qc):
                        # per-slot transfers: two 128KB DMAs land on two
                        # queues, halving the final drain vs one 256KB.
                        # Copies alternate ACT/DVE (both idle by now).
                        for i in range(2):
                            y_t = Cy.tile([128, QB], bf16, tag="yt")
                            if n_cp[0] % 2 == 0:
                                nc.scalar.activation(
                                    out=y_t,
                                    in_=ps[:, i],
                                    func=mybir.ActivationFunctionType.Copy,
                                    scale=1.0,
                                )
                            else:
                                nc.vector.tensor_copy(out=y_t, in_=ps[:, i])
                            n_cp[0] += 1
                            nc.sync.dma_start(
                                out=y_d.ap()[
                                    qc * 128 : (qc + 1) * 128,
                                    i * QB : (i + 1) * QB,
                                ],
                                in_=y_t,
                            )

                    # slots 4-7 go through the psJ ring first: no wait on
                    # the score-ring copies, so every close matmul runs
                    # back-to-back at the warm clock
                    qb = slots[0][0] // NQB
                    for k in (4, 5, 6, 7):
                        emit_outproj_one(qb, k, on_act=(k % 2 == 0))
                    for ps, grp in ss_tiles:
                        for i, (qc, mb) in enumerate(grp):
                            nc.tensor.matmul(
                                ps[:, i],
                                ot_t[:, 1, qc * 128 : (qc + 1) * 128],
                                w0_t[:, 1, mb * QB : (mb + 1) * QB],
                                start=False,
                                stop=True,
                            )
                        copy_out_row(ps, grp[0][0])

                def emit_step(
                    qb, hp, prev, ride=None, leftover=None, fillers=None,
                    final=False
                ):
                    """Scores+exp stream for (qb, hp); the previous step's
                    PV chunks are interleaved between score-group PAIRS
                    (blocks of 4 chunks, trailing by one block) so the
                    tensor queue streams without the psS/exp lockstep and
                    PV matmuls run back-to-back. `ride` is an out-proj
                    portion (qb, lo, hi) emitted mid-step. The final step
                    runs its leftovers after the score stream and chases
                    the exp tail with its own PV."""
                    q_sl = slice(qb * QB, (qb + 1) * QB)
                    expst = C.tile([128, NCH, QB], bf16, tag="expst")
                    if prev is not None:
                        pq, php, pexp = prev
                        ps_oa = psO.tile([128, QB], f32, tag="oa")
                        ps_ob = psO.tile([128, QB], f32, tag="ob")

                    def pv_prev(g0, g1):
                        for c in range(g0, g1):
                            kc, hb = c // 2, c % 2
                            ps_o = ps_oa if hb == 0 else ps_ob
                            nc.tensor.matmul(
                                ps_o[0 : DK + 1, :],
                                v_t[:, kc, 2 * php + hb, :],
                                pexp[:, c, :],
                                start=(kc == 0),
                                stop=(kc == NKC - 1),
                            )

                    def score_group(g0, g1):
                        ps = psS.tile([128, 2, QB], f32, tag="s", name="ps_s")
                        for i, c in enumerate(range(g0, g1)):
                            kc, hb = c // 2, c % 2
                            p0 = hb * 64
                            k_sl = slice(kc * 128, (kc + 1) * 128)
                            nc.tensor.matmul(
                                ps[:, i],
                                kt_t[p0 : p0 + 64, hp, k_sl],
                                qt_t[p0 : p0 + 64, hp, q_sl],
                                start=True,
                                stop=True,
                                tile_position=(p0, 0),
                            )
                        nc.scalar.activation(
                            out=expst[:, g0:g1, :],
                            in_=ps[:, 0 : g1 - g0, :],
                            func=mybir.ActivationFunctionType.Exp,
                            scale=SCALE_INV,
                        )

                    NBLK = len(GRPS) // 2  # 8 blocks of 2 groups
                    if not final:
                        for b in range(NBLK):
                            score_group(*GRPS[2 * b])
                            score_group(*GRPS[2 * b + 1])
                            if prev is not None and b >= 1:
                                pv_prev(4 * (b - 1), 4 * b)
                            if ride is not None and b in (1, 3, 5, 7):
                                # one out-proj slot of an already-normalized
                                # block rides per block, through the idle
                                # psJ ring: the score-PSUM ring is never
                                # displaced and the copy is off its path
                                emit_outproj_one(
                                    ride[0], ride[1] + (b - 1) // 2
                                )
                            if fillers:
                                fillers.pop(0)()
                            if fillers:
                                fillers.pop(0)()
                        if prev is not None:
                            pv_prev(4 * (NBLK - 1), NCH)
                            emit_normalize(pq, php, ps_oa, ps_ob)
                        return expst

                    # final step: prev-PV interleaved as usual; leftover
                    # out-proj rides after the score stream where it fills
                    # the window in which the tensor engine would otherwise
                    # wait for the exp tail before self-PV.
                    for b in range(NBLK):
                        score_group(*GRPS[2 * b])
                        score_group(*GRPS[2 * b + 1])
                        if prev is not None and b >= 1:
                            pv_prev(4 * (b - 1), 4 * b)
                    if prev is not None:
                        pv_prev(4 * (NBLK - 1), NCH)
                        emit_normalize(pq, php, ps_oa, ps_ob)
                    # leftover rides fill the exp-tail wait (they must
                    # fully precede the open jc=0 partials: both use the
                    # psS ring and an open accumulation may not be
                    # recycled)
                    for r in leftover or []:
                        emit_outproj(r[0], r[1], r[2], copies_on_act=True)
                    op_state = emit_outproj_final_open(qb)

                    ps_fa = psO.tile([128, QB], f32, tag="oa")
                    ps_fb = psO.tile([128, QB], f32, tag="ob")
                    for c in range(NCH):
                        kc, hb = c // 2, c % 2
                        ps_o = ps_fa if hb == 0 else ps_fb
                        nc.tensor.matmul(
                            ps_o[0 : DK + 1, :],
                            v_t[:, kc, 2 * hp + hb, :],
                            expst[:, c, :],
                            start=(kc == 0),
                            stop=(kc == NKC - 1),
                        )
                    emit_normalize(qb, hp, ps_fa, ps_fb)
                    # keep the PE p-state warm through the z chain --
                    # idle >1.5us drops the clock and the closing matmuls
                    # would run ~2x slower
                    warm_ps = psJ.tile([128, QB], f32, tag="pj", name="ps_pj")
                    for _ in range(6):
                        nc.tensor.matmul(
                            warm_ps,
                            kt_t[:, 0, 0:128],
                            kt_t[:, 0, 0:QB],
                            start=True,
                            stop=True,
                        )
                    # the last block's jc=1 halves close out after the
                    # final z chain
                    emit_outproj_final_close(*op_state)
                    return expst

                steps = [(qb, hp) for qb in range(NQB) for hp in range(2)]
                # Deferred projection fillers: V seq-chunk pairs 0-1 and
                # the remaining Q blocks ride step 0; V pairs 2-7 ride
                # step 1 (pair p lands before the prev-PV chunks that
                # read v_t[2p] with >=2 group slots of margin).
                step_fillers = {
                    0: [k_filler(0), k_filler(1), v_filler(0), v_filler(1)]
                    + [q_filler(jc, qb) for qb in (1, 2, 3) for jc in (0, 1)],
                    1: [v_filler(p) for p in range(2, 8)],
                }
                prev = None
                rides = []  # (qb, lo, hi) out-proj portions awaiting a step
                for i, (qb, hp) in enumerate(steps):
                    last = i == len(steps) - 1
                    expst = emit_step(
                        qb,
                        hp,
                        prev,
                        ride=(rides.pop(0) if rides and not last else None),
                        leftover=rides if last else None,
                        fillers=step_fillers.get(i),
                        final=last,
                    )
                    # prev (consumed this step) is normalized at this
                    # step's end; its out-proj rides later steps in halves.
                    if prev is not None and prev[1] == 1:
                        rides.append((prev[0], 0, 4))
                        rides.append((prev[0], 4, 8))
                    prev = (qb, hp, expst)

    nc.compile()
    return nc


def kernel(X, W_Q, W_K, W_V, W_0):
    global LAST_RESULT
    from concourse.bass_utils import run_bass_kernel_spmd
    import os

    bf = ml_dtypes.bfloat16
    X = np.asarray(X, dtype=np.float32)
    W_Q = np.asarray(W_Q, dtype=np.float32).astype(bf)
    W_K = np.asarray(W_K, dtype=np.float32).astype(bf)
    W_V = np.asarray(W_V, dtype=np.float32).astype(bf)
    W_0 = np.asarray(W_0, dtype=np.float32).astype(bf)

    if "nc" not in _CACHE:
        _CACHE["nc"] = _build()
    nc = _CACHE["nc"]

    xt = [np.ascontiguousarray(X[b].T).astype(bf) for b in range(B)]
    in_maps = []
    for c in range(NCORES):
        b, g = c // HG, c % HG
        js = slice(g * J, (g + 1) * J)
        in_maps.append(
            {
                "xt": xt[b],
                "wq": np.ascontiguousarray(W_Q[:, js]),
                "wk": np.ascontiguousarray(W_K[:, js]),
                "wv": np.ascontiguousarray(W_V[:, js]),
                "w0": np.ascontiguousarray(W_0[js, :]),
            }
        )

    trace = bool(int(os.environ.get("KERNEL_TRACE", "0")))
    res = run_bass_kernel_spmd(
        nc, in_maps, list(range(NCORES)), trace=trace
    )
    LAST_RESULT = res

    out = np.zeros((B, S, D), dtype=np.float32)
    for c in range(NCORES):
        out[c // HG] += res.results[c]["y"].astype(np.float32)
    return out

